# revision 19
# baseline (speedup 1.0000x reference)
"""Trainium2 Bass kernel: causal multi-head self-attention with RoPE.

Problem: x[2,2048,1024], 16 heads, d_k=64, causal, RoPE(theta=1e4),
out = (softmax(rope(Q)rope(K)^T/8) V) WO^T.

Sharding (8 cores): data-parallel over batch (2) x head-parallel over
head groups (4 heads per core).  Each core computes Q/K/V projections
for its 4 heads, causal attention, and a partial output projection over
its 256 channels; the host sums the 4 partials per batch element.

v2 design (ACT-exp is the bottleneck engine: ~8.4M exp elements/core):
  j-pipelined emission: projections for seq-slice j interleave with
      attention for query-slice j so the scalar engine's exp stream
      starts ~5us into the kernel and everything else hides under it.
  scores computed transposed ([keys,queries]); the two heads of a pair
      live at partitions 0-63 / 64-127, so back-to-back score matmuls
      (C=64) land in disjoint row-groups and run concurrently
      (hardware row tiling via auto tile_position).
  RoPE: quadrant-local even/odd layout (host permutes W_Q/W_K columns)
      so the rotate-half partner swap is a DVE stream_shuffle
      (within-32-partition), no DMAs.
  diagonal causal masking: exp the full block, then zero the triangle
      on GpSimd with a 0/1 multiply on the exp output (keeps PE and
      ACT free of masking work).
  V carries a 65th column of ones => PV yields softmax denominators.
  softmax normalization deferred: unnormalized head outputs + den rows
      staged, then reciprocal + indicator-matmul broadcast rescale in
      a tail fused with the output projection.
  PSUM: one shared [128,1024] pool (bufs=3, 6 banks) for QK-proj,
      V-proj and scores; 2 banks for PV accumulators. Tail pools open
      after the attention scope closes.
"""

import os
import sys

for _p in ("/opt/trn_rl_repo",):
    if _p not in sys.path:
        sys.path.insert(0, _p)

import numpy as np
import ml_dtypes

BF16 = ml_dtypes.bfloat16

D = 1024
S = 2048
H = 16
DK = 64
HPC = 4          # heads per core
NCORES = 8
THETA = 10000.0

_COMPILED = {}
_SENT = object()


def _build_nc():
    import concourse.bass as bass  # noqa: F401
    import concourse.bacc as bacc
    import concourse.mybir as mybir
    import concourse.tile as tile

    bf16 = mybir.dt.bfloat16
    f32 = mybir.dt.float32
    Exp = mybir.ActivationFunctionType.Exp

    nc = bacc.Bacc(
        "TRN2", target_bir_lowering=False, debug=False, num_devices=NCORES
    )
    xt_d = nc.declare_dram_parameter("xt", [8, 128, 4, 512], bf16, isOutput=False)
    wq_d = nc.declare_dram_parameter("wq", [128, 8, 256], bf16, isOutput=False)
    wk_d = nc.declare_dram_parameter("wk", [128, 8, 256], bf16, isOutput=False)
    wv_d = nc.declare_dram_parameter("wv", [128, 8, 256], bf16, isOutput=False)
    wo_d = nc.declare_dram_parameter("wo", [128, 2, D], bf16, isOutput=False)
    cos_d = nc.declare_dram_parameter("cos2", [128, 4, 1024], bf16, isOutput=False)
    sin_d = nc.declare_dram_parameter("sin2", [128, 4, 1024], bf16, isOutput=False)
    tri_d = nc.declare_dram_parameter("tri3", [128, 768], bf16, isOutput=False)
    ind_d = nc.declare_dram_parameter("ind", [40, 4, 128], bf16, isOutput=False)
    out_d = nc.declare_dram_parameter("out", [S, D], bf16, isOutput=True)

    SHUF = [(i + 16) % 32 for i in range(32)]  # rotate-half partner swap

    with tile.TileContext(nc) as tc:
        with tc.tile_pool(name="const", bufs=1) as const:
            x_sb = const.tile([128, 8, S], bf16)
            wq_sb = const.tile([128, 8, 256], bf16)
            wk_sb = const.tile([128, 8, 256], bf16)
            wv_sb = const.tile([128, 8, 256], bf16)
            wo_sb = const.tile([128, 2, D], bf16)
            cos_sb = const.tile([128, 4, 1024], bf16)
            sin_sb = const.tile([128, 4, 1024], bf16)
            tri_sb = const.tile([128, 768], bf16)
            ind_sb = const.tile([40, 4, 128], bf16)
            v_sb = const.tile([128, 16, 4, 65], bf16)
            # qkrot[ot][:, nsl, 0:512] = rope(Q) slice, [:, nsl, 512:1024] = rope(K)
            qkrot = [const.tile([128, 4, 1024], bf16, name=f"qkrot{i}")
                     for i in range(2)]
            at = [const.tile([128, S], bf16, name=f"at{i}") for i in range(2)]
            den_sb = const.tile([40, 512], bf16)
            denf = const.tile([40, 512], f32)
            rc = const.tile([40, 512], f32)
            rcb = const.tile([40, 512], bf16)
            atn = [const.tile([128, 4, 512], bf16, name=f"atn{i}")
                   for i in range(2)]

            # x slice halves alternate across the two DMA queues so slice 0
            # lands fast; weights stream on the gpsimd queue, first-needed
            # first
            for nsl in range(4):
                nc.sync.dma_start(
                    x_sb[:, 0:4, nsl * 512:(nsl + 1) * 512], xt_d[2 * nsl]
                )
            nc.gpsimd.dma_start(x_sb[:, 4:8, 0:512], xt_d[1])
            nc.gpsimd.dma_start(wq_sb[:], wq_d[:])
            nc.gpsimd.dma_start(wk_sb[:], wk_d[:])
            nc.gpsimd.dma_start(wv_sb[:], wv_d[:])
            nc.gpsimd.dma_start(cos_sb[:, 0], cos_d[:, 0])
            nc.gpsimd.dma_start(sin_sb[:, 0], sin_d[:, 0])
            nc.gpsimd.dma_start(tri_sb[:], tri_d[:])
            for nsl in range(1, 4):
                nc.gpsimd.dma_start(
                    x_sb[:, 4:8, nsl * 512:(nsl + 1) * 512], xt_d[2 * nsl + 1]
                )
                nc.gpsimd.dma_start(cos_sb[:, nsl], cos_d[:, nsl])
                nc.gpsimd.dma_start(sin_sb[:, nsl], sin_d[:, nsl])
            nc.gpsimd.dma_start(ind_sb[:], ind_d[:])
            nc.gpsimd.dma_start(wo_sb[:], wo_d[:])
            nc.vector.memset(v_sb[:, :, :, 64:65], 1.0)
            # den rows for later j are read (masked by ind=0) by early tail
            # reciprocals before they are staged: keep them finite
            nc.vector.memset(den_sb[:], 1.0)
            # load the Exp activation table before the first real exp needs it
            warm = const.tile([1, 16], f32)
            warmo = const.tile([1, 16], bf16)
            nc.vector.memset(warm[:], 0.0)
            nc.scalar.activation(warmo[:], warm[:], Exp)

            with tc.tile_pool(name="ps", bufs=3, space="PSUM") as psp, \
                 tc.tile_pool(name="po", bufs=2, space="PSUM") as pop, \
                 tc.tile_pool(name="rp", bufs=3) as rp, \
                 tc.tile_pool(name="pt", bufs=4) as ptp, \
                 tc.tile_pool(name="tm", bufs=3) as tmp:

                def gen_v(nsl):
                    # V for key blocks 4nsl..4nsl+3 (all 4 heads), one psum
                    # tile; yields every ~8 matmuls so it can interleave into
                    # attention groups (keeps the PE dense => HAM warm)
                    ps = psp.tile([128, 1024], f32, tag="ps", name="psv")
                    for sbl in range(4):
                        sb = 4 * nsl + sbl
                        for c in range(8):
                            nc.tensor.matmul(
                                ps[:, sbl * 256:(sbl + 1) * 256],
                                x_sb[:, c, sb * 128:(sb + 1) * 128],
                                wv_sb[:, c, :],
                                start=(c == 0), stop=(c == 7),
                            )
                        yield
                    nc.vector.tensor_copy(
                        v_sb[:, 4 * nsl:4 * nsl + 4, :, 0:64],
                        ps[:].rearrange("p (s h d) -> p s h d", s=4, h=4),
                    )
                    yield

                def gen_qk(ot, nsl):
                    # Q (cols 0:512) + K (cols 512:1024) for head pair ot,
                    # seq slice nsl; then rope into qkrot[ot][:, nsl, :]
                    w_pair = ((wq_sb, 0), (wk_sb, 512))
                    ps = psp.tile([128, 1024], f32, tag="ps", name="psqk")
                    for w_sb, off in w_pair:
                        for ch in range(2):
                            for c in range(4 * ch, 4 * ch + 4):
                                nc.tensor.matmul(
                                    ps[:, off:off + 512],
                                    w_sb[:, c, ot * 128:(ot + 1) * 128],
                                    x_sb[:, c, nsl * 512:(nsl + 1) * 512],
                                    start=(c == 0), stop=(c == 7),
                                )
                            yield
                    raw = rp.tile([128, 1024], bf16, tag="raw", name="raw")
                    shf = rp.tile([128, 1024], bf16, tag="shf", name="shf")
                    t1 = rp.tile([128, 1024], bf16, tag="t1", name="t1")
                    nc.vector.tensor_copy(raw[:], ps[:])
                    nc.vector.stream_shuffle(shf[:], raw[:], SHUF)
                    # rot = raw*cos + shuffle(raw)*sin (sin sign is
                    # destination-indexed, folded into sin_sb)
                    nc.vector.tensor_mul(t1[:], raw[:], cos_sb[:, nsl])
                    nc.vector.tensor_mul(shf[:], shf[:], sin_sb[:, nsl])
                    nc.vector.tensor_add(qkrot[ot][:, nsl], t1[:], shf[:])
                    yield

                def chain(*gens):
                    for g in gens:
                        yield from g

                def gen_tail(jsl):
                    # normalization + output projection for query slice jsl;
                    # runs as attention filler (jsl<3) or at the end (jsl=3)
                    nc.vector.tensor_copy(denf[:], den_sb[:])
                    nc.vector.reciprocal_approx_fast(rc[:], denf[:])
                    nc.vector.tensor_copy(rcb[:], rc[:])
                    yield
                    ps = psp.tile([128, 1024], f32, tag="ps", name="psrb")
                    for ot in range(2):
                        nc.tensor.matmul(
                            ps[:, ot * 512:(ot + 1) * 512],
                            ind_sb[ot * 32:ot * 32 + 8, jsl, :],
                            rcb[ot * 32:ot * 32 + 8, :],
                            start=True, stop=True,
                        )
                    for ot in range(2):
                        nc.vector.tensor_mul(
                            atn[ot][:, jsl, :],
                            at[ot][:, jsl * 512:(jsl + 1) * 512],
                            ps[:, ot * 512:(ot + 1) * 512],
                        )
                    yield
                    for sbi in range(4):
                        sb = jsl * 4 + sbi
                        ps2 = psp.tile([128, 1024], f32, tag="ps", name="psof")
                        ob = ptp.tile([128, 1024], bf16, tag="pt", name="ob")
                        for osl in range(2):
                            for ich in range(2):
                                nc.tensor.matmul(
                                    ps2[:, osl * 512:(osl + 1) * 512],
                                    atn[ich][:, jsl, sbi * 128:(sbi + 1) * 128],
                                    wo_sb[:, ich, osl * 512:(osl + 1) * 512],
                                    start=(ich == 0), stop=(ich == 1),
                                )
                        nc.vector.tensor_copy(ob[:], ps2[:])
                        nc.sync.dma_start(
                            out_d[sb * 128:(sb + 1) * 128, :], ob[:]
                        )
                        yield

                def attention(ot, j, filler=None, fsteps=0):
                    nkb = 4 * (j + 1)
                    ngrp = nkb // 2
                    fdone = 0
                    po = [pop.tile([65, 512], f32, tag="po", name="po")
                          for _ in range(2)]
                    for g in range(nkb // 2):
                        kb0 = 2 * g
                        dg0 = kb0 - 4 * j  # >= 0 on diagonal groups
                        sp = [psp.tile([128, 1024], f32, tag="ps", name="sp")
                              for _ in range(2)]
                        for i in range(2):
                            kb = kb0 + i
                            isl, ioff = kb // 4, (kb % 4) * 128
                            c0 = max(0, kb - 4 * j) * 128
                            for hl in range(2):  # row-tiled pair: concurrent
                                r0 = hl * 64
                                nc.tensor.matmul(
                                    sp[hl][:, i * 512 + c0:(i + 1) * 512],
                                    qkrot[ot][r0:r0 + 64, isl,
                                              512 + ioff:512 + ioff + 128],
                                    qkrot[ot][r0:r0 + 64, j, c0:512],
                                    start=True, stop=True,
                                )
                        e0 = max(0, dg0) * 128  # exp skips fully-masked lead
                        for hl in range(2):
                            pt = ptp.tile([128, 1024], bf16, tag="pt", name="pt")
                            nc.scalar.activation(
                                pt[:, e0:1024], sp[hl][:, e0:1024],
                                Exp, scale=0.125,
                            )
                            if dg0 >= 0:
                                # zero the two 128-col causal triangles
                                # (cols [e0,e0+128) and [e0+640,e0+768))
                                nc.vector.tensor_mul(
                                    pt[:, e0:e0 + 768], pt[:, e0:e0 + 768],
                                    tri_sb[:],
                                )
                            for i in range(2):
                                kb = kb0 + i
                                c0 = max(0, kb - 4 * j) * 128
                                nc.tensor.matmul(
                                    po[hl][:, c0:512],
                                    v_sb[:, kb, 2 * ot + hl, 0:65],
                                    pt[:, i * 512 + c0:(i + 1) * 512],
                                    start=(kb == 0), stop=(kb == nkb - 1),
                                )
                        if filler is not None:
                            want = (fsteps * (g + 1) + ngrp - 1) // ngrp
                            while fdone < want:
                                if next(filler, _SENT) is _SENT:
                                    filler = None
                                    break
                                fdone += 1
                    while filler is not None and next(filler, _SENT) is not _SENT:
                        pass
                    for hl in range(2):
                        tm = tmp.tile([65, 512], bf16, tag="tm", name="tm")
                        nc.vector.tensor_copy(tm[:], po[hl][:])
                        nc.sync.dma_start(
                            at[ot][hl * 64:hl * 64 + 64,
                                   j * 512:(j + 1) * 512],
                            tm[0:64, :],
                        )
                        dr = ot * 32 + hl * 4 + j
                        nc.sync.dma_start(den_sb[dr:dr + 1, :], tm[64:65, :])

                # staggered schedule: att(ot, j) runs with the projection
                # work for the next unit interleaved as filler (one live
                # proj psum tile at a time; inputs always one unit ahead)
                for _ in gen_qk(0, 0):
                    pass
                for _ in gen_v(0):
                    pass
                for j in range(4):
                    f0, n0 = gen_qk(1, j), 5
                    if j < 3:
                        f1, n1 = chain(gen_qk(0, j + 1), gen_v(j + 1)), 10
                    else:
                        # last unit has ACT slack and no projections left:
                        # fold the tail for slices 0-2 in as filler
                        f1, n1 = chain(*[gen_tail(s) for s in range(3)]), 18
                    attention(0, j, f0, n0)
                    attention(1, j, f1, n1)
                for _ in gen_tail(3):
                    pass
    nc.compile()
    return nc


def _host_prep(x, token_positions, WQ, WK, WV, WO):
    """Build the 8 per-core input maps."""
    pos = np.asarray(token_positions).astype(np.float32)
    k = np.arange(DK // 2, dtype=np.float32)
    inv_freq = 1.0 / (THETA ** (2.0 * k / DK))
    ang = pos[:, None] * inv_freq[None, :]          # [S, 32]
    cosk = np.cos(ang).T.astype(np.float32)         # [32, S] (row=k)
    sink = np.sin(ang).T.astype(np.float32)

    # quadrant-local even/odd layout per 64-row head-half:
    # rows 0-15: dims 0,2,..,30 (k=0..15)   rows 16-31: dims 1,3,..,31
    # rows 32-47: dims 32,..,62 (k=16..31)  rows 48-63: dims 33,..,63
    perm64 = np.concatenate([
        np.arange(0, 32, 2), np.arange(1, 32, 2),
        np.arange(32, 64, 2), np.arange(33, 64, 2),
    ])
    krow = np.concatenate([k[0:16], k[0:16], k[16:32], k[16:32]]).astype(int)
    # cos row pattern [64]; rot = raw*cos + shuffle(raw)*sin
    # sin multiplies AFTER the shuffle, so the sign is destination-indexed:
    # even-rows (r%32<16) get -sin (r1 = x1 cos - x2 sin), odd-rows +sin
    sgn = np.where((np.arange(64) % 32) < 16, -1.0, 1.0).astype(np.float32)
    cos64 = cosk[krow, :]                           # [64, S]
    sin64 = sink[krow, :] * sgn[:, None]            # [64, S]
    cos128 = np.concatenate([cos64, cos64], axis=0)  # [128, S]
    sin128 = np.concatenate([sin64, sin64], axis=0)
    # [128, 4, 1024]: per slice, duplicated for the Q|K halves
    cos2 = np.stack([np.concatenate(
        [cos128[:, s * 512:(s + 1) * 512]] * 2, axis=1) for s in range(4)],
        axis=1).astype(BF16)
    sin2 = np.stack([np.concatenate(
        [sin128[:, s * 512:(s + 1) * 512]] * 2, axis=1) for s in range(4)],
        axis=1).astype(BF16)

    # tri3 [128, 768]: [tri01 | ones 512 | tri01] ; tri01[k,t] = k<=t
    kk = np.arange(128)
    tri01 = (kk[:, None] <= kk[None, :]).astype(np.float32)
    tri3 = np.concatenate(
        [tri01, np.ones((128, 512), np.float32), tri01], axis=1).astype(BF16)

    # indicator matrices for denominator broadcast:
    # ind[i, jsl, r] = 1 iff i == ot*32 + (r//64)*4 + jsl
    ind = np.zeros((40, 4, 128), dtype=np.float32)
    for jsl in range(4):
        for r in range(128):
            ind[(r // 64) * 4 + jsl, jsl, r] = 1.0
            ind[32 + (r // 64) * 4 + jsl, jsl, r] = 1.0
    ind = ind.astype(BF16)

    in_maps = []
    for core in range(NCORES):
        b, hg = divmod(core, 4)
        ch0 = hg * 256
        qk_rows = np.concatenate([ch0 + hl * 64 + perm64 for hl in range(HPC)])

        def dev_w(w):  # [D, M] -> [128, 8, M] (contraction chunks)
            return np.ascontiguousarray(
                w.reshape(8, 128, -1).transpose(1, 0, 2)
            ).astype(BF16)

        xt = np.asarray(x[b]).T                       # [D, S]
        # xt_d[2*nsl+h][pc, cc, s] = xt[(4h+cc)*128+pc, nsl*512+s]
        xt4 = np.ascontiguousarray(
            xt.reshape(2, 4, 128, 4, 512).transpose(3, 0, 2, 1, 4)
            .reshape(8, 128, 4, 512)
        ).astype(BF16)
        in_maps.append({
            "xt": xt4,
            "wq": dev_w(np.asarray(WQ)[qk_rows, :].T),
            "wk": dev_w(np.asarray(WK)[qk_rows, :].T),
            "wv": dev_w(np.asarray(WV)[ch0:ch0 + 256, :].T),
            "wo": np.ascontiguousarray(
                np.asarray(WO)[:, ch0:ch0 + 256].T.reshape(2, 128, D)
                .transpose(1, 0, 2)
            ).astype(BF16),
            "cos2": cos2,
            "sin2": sin2,
            "tri3": tri3,
            "ind": ind,
        })
    return in_maps


LAST_EXEC_NS = None
LAST_RESULT = None


def kernel(x, token_positions, WQ, WK, WV, WO):
    global LAST_EXEC_NS, LAST_RESULT
    from concourse.bass_utils import run_bass_kernel_spmd

    if "nc" not in _COMPILED:
        _COMPILED["nc"] = _build_nc()
    nc = _COMPILED["nc"]

    in_maps = _host_prep(x, token_positions, WQ, WK, WV, WO)
    res = run_bass_kernel_spmd(nc, in_maps, list(range(NCORES)))
    LAST_EXEC_NS = res.exec_time_ns
    LAST_RESULT = res

    out = np.zeros((2, S, D), dtype=np.float32)
    for core in range(NCORES):
        out[core // 4] += np.asarray(res.results[core]["out"], dtype=np.float32)
    return out
